# revision 28
# baseline (speedup 1.0000x reference)
"""Trainium2 Bass kernel for nn_ClassificationLayer (Gaussian pdf-sum classifier).

Math:
  mu/sd per dim from tiny [128,10] reference sets (host, exact).
  Per row i: s_n[i] = sum_d INV_SQRT_2PI/sd_d * exp(-0.5*((x[i,d]-mu_d)/sd_d)^2)
  (same for anomaly), then the batch recurrence p_k = (p_{k-1} + s_k)/128,
  output = [pn/(pn+pa), pa/(pn+pa)].

Device strategy (8 cores, data-parallel over N, 62500 rows/core exactly):
  - Host transposes each core's row-shard to fp16 [128 dims, R rows]; per-dim
    constants become per-partition scale/bias for the ScalarEngine. fp16 input
    halves HBM traffic so the DMA stream (45us) hides under the ScalarEngine
    floor (2 activation passes over all elements ~ 105us).
  - One ACTIVATE per distribution per tile: Derivative_Erf(scale*x + bias)
    = (2/sqrt(pi)) * exp(-((x-mu)/sd)^2/2) -- the whole Gaussian in one pass.
    The framework auto-inserts the activation-table load right after the
    preamble, off the critical path.
  - Reduction over dims (partitions) via TensorEngine matvec (fp16 x fp16).
    The stationary operand is a 64-wide shifted window over a zero-padded
    fp16 weight buffer so chunk g's sums land in PSUM partition g%64 of bank
    g//64. Chunk 121 and the 36-col runt use 1-wide stationaries into tiny
    2-partition PSUM banks so the final drain is one short copy + one DMA.
  - Output DRAM rows are padded to 128KB stride: the simulated SDMA assigns
    engine = (dram_offset / 128KB) % 16, so 4KB-strided rows would all land
    on 2 of the 16 engines and serialize the drain.
  - The scalar recurrence decays by 1/128 per step, so it is re-run exactly
    on the gathered per-row sums on host as a short causal convolution.
"""

import numpy as np

N, DIM, S = 500000, 128, 10
INV_SQRT_2PI = 0.3989422804014327
NCORES = 8
R = N // NCORES                  # 62500 rows per core, exact
CHUNK = 512                      # rows per matvec (PSUM bank free-dim)
FULL_CHUNKS = R // CHUNK         # 122
RUNT = R - FULL_CHUNKS * CHUNK   # 36
# tile widths in chunks: small head tiles so the ScalarEngine starts before
# the first big DMA lands, wide tiles to amortize per-instruction overhead,
# small tail so the final drain chain is short. The runt's 36 cols ride on
# the last tile (548-col activations, separate matmul).
TILE_CHUNKS = [1, 2, 4, 8, 16, 24, 24, 24, 16, 2, 1]
assert sum(TILE_CHUNKS) == FULL_CHUNKS
MAX_W = max(TILE_CHUNKS) * CHUNK
PADW = 32768                     # fp32 cols per padded DRAM out row (128KB)

_COMPILED = None
LAST_RESULTS = None  # BassKernelResults of the most recent device run


def _build():
    import concourse.tile as tile
    from concourse import bacc, mybir

    nc = bacc.Bacc("TRN2", target_bir_lowering=False, debug=False,
                   num_devices=NCORES)

    xT = nc.dram_tensor("xT", [DIM, R], mybir.dt.float16,
                        kind="ExternalInput").ap()
    # consts: col 0 scale_n, 1 bias_n, 2 scale_a, 3 bias_a,
    #         cols 4-7 = cols 0-3 / 4 (DVE chain: z' = z/sqrt(32))
    consts = nc.dram_tensor("consts", [DIM, 8], mybir.dt.float32,
                            kind="ExternalInput").ap()
    # wmat: col 128-r of window [128-r, 192-r) must be the weight vector:
    # c_n at col 128, c_a at col 384; everything else exactly 0.
    wmat = nc.dram_tensor("wmat", [DIM, 512], mybir.dt.float16,
                          kind="ExternalInput").ap()
    # outA: bank A (chunks 0-63):   rows 0-63, [sn 512 | sa 512]
    # outB: bank B (chunks 64-120): rows 0-56 (57-63 zero), [sn 512 | sa 512]
    #   (64 rows, not 57: the sim spreads a DMA over
    #    largest-divisor-of-partition-count <= 16 engines, so 57 rows would
    #    crawl on 3 engines while 64 rows get all 16)
    # outC: chunk 121 + runt:       one row [sn 512 | sn 36 | sa 512 | sa 36]
    outA = nc.dram_tensor("outA", [64, PADW], mybir.dt.float32,
                          kind="ExternalOutput").ap()
    outB = nc.dram_tensor("outB", [64, PADW], mybir.dt.float32,
                          kind="ExternalOutput").ap()
    outC = nc.dram_tensor("outC", [1, PADW], mybir.dt.float32,
                          kind="ExternalOutput").ap()

    DErf = mybir.ActivationFunctionType.Derivative_Erf

    tile_first_chunk = []
    g = 0
    for tc_ in TILE_CHUNKS:
        tile_first_chunk.append(g)
        g += tc_
    FLUSH_A_TILE = next(i for i, f in enumerate(tile_first_chunk)
                        if f + TILE_CHUNKS[i] > 63)
    FLUSH_B_TILE = next(i for i, f in enumerate(tile_first_chunk)
                        if f + TILE_CHUNKS[i] > 120)
    LAST_TILE = len(TILE_CHUNKS) - 1

    with tile.TileContext(nc) as tc, \
         nc.allow_low_precision(reason="fp16 pdf terms; sums accumulate in fp32 PSUM"):
        with tc.tile_pool(name="cpool", bufs=1) as cpool, \
             tc.tile_pool(name="xpool", bufs=3) as xpool, \
             tc.tile_pool(name="epool", bufs=2) as epool, \
             tc.tile_pool(name="dpool", bufs=1) as dpool, \
             tc.tile_pool(name="pspool", bufs=1, space="PSUM") as pspool:

            # first x tile ahead of the (tiny) consts DMA: x0's transfer
            # dominates the first ACTIVATE's critical path
            consts_t = cpool.tile([DIM, 8], mybir.dt.float32)
            x_pre = {}
            for ti in (0, 1):
                w = TILE_CHUNKS[ti] * CHUNK
                off = tile_first_chunk[ti] * CHUNK
                x_t = xpool.tile([DIM, w], mybir.dt.float16, tag="x",
                                 padded_shape=[DIM, MAX_W],
                                 name=f"x_pre{ti}")
                nc.sync.dma_start(x_t[:], xT[:, off:off + w])
                x_pre[ti] = x_t
            nc.sync.dma_start(consts_t[:], consts[:, :])
            # Dummy activation gated only on the tiny consts DMA: without it
            # the auto-inserted table load attaches to the first real ACTIVATE
            # and ends up waiting on the x-tile DMA, delaying the stream.
            warm_t = cpool.tile([DIM, 1], mybir.dt.float32)
            nc.scalar.activation(warm_t[:], consts_t[:, 0:1], DErf,
                                 bias=0.0, scale=1.0)
            # weights via SWDGE so the Sync HWDGE queue is x-tiles only
            w_t = cpool.tile([DIM, 512], mybir.dt.float16)
            nc.gpsimd.dma_start(w_t[:], wmat[:, :])

            # per dist: bank A = chunks 0-63, bank B = chunks 64-120,
            # bank C = chunk 121 (row 0 sn, row 1 sa), bank D = runt
            sn_psA = pspool.tile([64, CHUNK], mybir.dt.float32)
            sn_psB = pspool.tile([64, CHUNK], mybir.dt.float32)
            sa_psA = pspool.tile([64, CHUNK], mybir.dt.float32)
            sa_psB = pspool.tile([64, CHUNK], mybir.dt.float32)
            # chunk 121 and runt: one single-row bank per dist (PE output
            # base partition must be 0/32/64, so they can't share a bank)
            psC_n = pspool.tile([1, CHUNK], mybir.dt.float32)
            psC_a = pspool.tile([1, CHUNK], mybir.dt.float32)
            psD_n = pspool.tile([1, RUNT], mybir.dt.float32)
            psD_a = pspool.tile([1, RUNT], mybir.dt.float32)

            stA = dpool.tile([64, 1024], mybir.dt.float32)
            stB = dpool.tile([64, 1024], mybir.dt.float32)
            # one partition row: [sn 512 | sn 36 | sa 512 | sa 36]
            stC = dpool.tile([1, 2 * (CHUNK + RUNT)], mybir.dt.float32)

            # DVE offload: for the big tiles the trailing DVE_W columns of
            # both distributions are computed on the otherwise-idle Vector
            # engine via exp(-u) ~ (deg-4 Taylor of exp(-u/16))^16, u clamped
            # at 12 (floor exp(-12), the tail contributes nothing anyway).
            # The two distributions' chains are interleaved step by step so
            # the ~400ns cross-instruction semaphore latency of one chain is
            # hidden under the other chain's execution.
            # heavier offload early, tapering at the end so the Vector engine
            # re-converges with the ScalarEngine before the drain tiles
            DVE_W = {3: 512, 4: 1024, 5: 1536, 6: 1536, 7: 1024, 8: 512}
            DVMAX = max(DVE_W.values())
            dvpool_tiles = [[dpool.tile([DIM, DVMAX], mybir.dt.float16,
                                        name=f"dv{j}_{k}") for j in range(3)]
                            for k in range(2)]
            MUL = mybir.AluOpType.mult
            ADD = mybir.AluOpType.add
            MIN = mybir.AluOpType.min
            C16 = float((2.0 / np.sqrt(np.pi)) ** (1.0 / 16.0))

            def dve_chains(x_t, aw, w, en_t, ea_t):
                dvw = w - aw
                xs = x_t[:, aw:w]
                ab = []
                for k, e_t in ((0, en_t), (1, ea_t)):
                    z_t, v_t, h_t = dvpool_tiles[k]
                    zs = z_t[:, 0:dvw]
                    vs = v_t[:, 0:dvw]
                    hs = h_t[:, 0:dvw]
                    sc = consts_t[:, 4 + 2 * k:5 + 2 * k]
                    bi = consts_t[:, 5 + 2 * k:6 + 2 * k]
                    ab.append((zs, vs, hs, sc, bi, e_t))
                steps = [
                    lambda zs, vs, hs, sc, bi, e:
                        nc.vector.tensor_scalar(zs, xs, sc, bi, MUL, ADD),
                    lambda zs, vs, hs, sc, bi, e:
                        nc.vector.tensor_tensor(vs, zs, zs, MUL),
                    lambda zs, vs, hs, sc, bi, e:
                        nc.vector.tensor_scalar(zs, vs, 0.75, None, MIN),
                    lambda zs, vs, hs, sc, bi, e:
                        nc.vector.tensor_scalar(hs, zs, 1.0 / 24, -1.0 / 6,
                                                MUL, ADD),
                    lambda zs, vs, hs, sc, bi, e:
                        nc.vector.tensor_tensor(vs, hs, zs, MUL),
                    lambda zs, vs, hs, sc, bi, e:
                        nc.vector.tensor_scalar(hs, vs, 0.5, None, ADD),
                    lambda zs, vs, hs, sc, bi, e:
                        nc.vector.tensor_tensor(vs, hs, zs, MUL),
                    lambda zs, vs, hs, sc, bi, e:
                        nc.vector.tensor_scalar(hs, vs, -1.0, None, ADD),
                    lambda zs, vs, hs, sc, bi, e:
                        nc.vector.tensor_tensor(vs, hs, zs, MUL),
                    # p0 scaled by (2/sqrt(pi))^(1/16) so p0^16 picks up the
                    # Derivative_Erf prefactor the weights divide out
                    lambda zs, vs, hs, sc, bi, e:
                        nc.vector.tensor_scalar(hs, vs, 1.0, C16, ADD, MUL),
                    lambda zs, vs, hs, sc, bi, e:
                        nc.vector.tensor_tensor(vs, hs, hs, MUL),
                    lambda zs, vs, hs, sc, bi, e:
                        nc.vector.tensor_tensor(hs, vs, vs, MUL),
                    lambda zs, vs, hs, sc, bi, e:
                        nc.vector.tensor_tensor(vs, hs, hs, MUL),
                    lambda zs, vs, hs, sc, bi, e:
                        nc.vector.tensor_tensor(e[:, aw:w], vs, vs, MUL),
                ]
                for step in steps:
                    for args in ab:
                        step(*args)

            g = 0
            for ti, tcn in enumerate(TILE_CHUNKS):
                w = tcn * CHUNK
                off = tile_first_chunk[ti] * CHUNK
                if ti == LAST_TILE:
                    w += RUNT     # runt columns ride on the last tile's acts
                if ti in x_pre:
                    x_t = x_pre[ti]
                else:
                    x_t = xpool.tile([DIM, w], mybir.dt.float16, tag="x",
                                     padded_shape=[DIM, MAX_W])
                    nc.sync.dma_start(x_t[:], xT[:, off:off + w])
                dvw = DVE_W.get(ti, 0)
                aw = w - dvw
                en_t = epool.tile([DIM, w], mybir.dt.float16, tag="en",
                                  padded_shape=[DIM, MAX_W])
                nc.scalar.activation(en_t[:, 0:aw], x_t[:, 0:aw], DErf,
                                     bias=consts_t[:, 1:2],
                                     scale=consts_t[:, 0:1])
                ea_t = epool.tile([DIM, w], mybir.dt.float16, tag="ea",
                                  padded_shape=[DIM, MAX_W])
                nc.scalar.activation(ea_t[:, 0:aw], x_t[:, 0:aw], DErf,
                                     bias=consts_t[:, 3:4],
                                     scale=consts_t[:, 2:3])
                if dvw:
                    dve_chains(x_t, aw, w, en_t, ea_t)
                for c in range(tcn):
                    sl = slice(c * CHUNK, (c + 1) * CHUNK)
                    r = g % 64
                    if g == 121:
                        # chunk 121: 1-wide stationaries into banks C
                        nc.tensor.matmul(psC_n[:], w_t[:, 128:129],
                                         en_t[:, sl], start=True, stop=True,
                                         skip_group_check=True)
                        nc.tensor.matmul(psC_a[:], w_t[:, 384:385],
                                         ea_t[:, sl], start=True, stop=True,
                                         skip_group_check=True)
                    else:
                        sn_ps = sn_psA if g < 64 else sn_psB
                        sa_ps = sa_psA if g < 64 else sa_psB
                        first = r == 0
                        last = g == 63 or g == 120
                        nc.tensor.matmul(sn_ps[:], w_t[:, 128 - r:192 - r],
                                         en_t[:, sl], start=first, stop=last,
                                         skip_group_check=True)
                        nc.tensor.matmul(sa_ps[:], w_t[:, 384 - r:448 - r],
                                         ea_t[:, sl], start=first, stop=last,
                                         skip_group_check=True)
                    g += 1
                if ti == LAST_TILE:
                    # runt: 36 cols through 1-wide stationaries into banks D
                    rsl = slice(tcn * CHUNK, tcn * CHUNK + RUNT)
                    nc.tensor.matmul(psD_n[:], w_t[:, 128:129],
                                     en_t[:, rsl], start=True, stop=True,
                                     skip_group_check=True)
                    nc.tensor.matmul(psD_a[:], w_t[:, 384:385],
                                     ea_t[:, rsl], start=True, stop=True,
                                     skip_group_check=True)
                if ti == FLUSH_A_TILE:
                    # bank A complete: drain it under the remaining compute
                    nc.vector.tensor_copy(stA[:, 0:512], sn_psA[:])
                    nc.vector.tensor_copy(stA[:, 512:1024], sa_psA[:])
                    nc.sync.dma_start(outA[:, 0:1024], stA[:])
                if ti == FLUSH_B_TILE:
                    # bank B rows 0-56 complete (57-63 zeroed by the start
                    # matmul): drain all 64 rows under the last tiles
                    nc.vector.tensor_copy(stB[:, 0:512], sn_psB[:])
                    nc.vector.tensor_copy(stB[:, 512:1024], sa_psB[:])
                    nc.sync.dma_start(outB[:, 0:1024], stB[:])

            # tail: short copies (sn on DVE, sa on the now-idle ScalarE in
            # parallel; GpSimd cannot read PSUM; engine writes must start at
            # partition 0/32/64/96 so everything lands in one partition row)
            CR = CHUNK + RUNT
            nc.vector.tensor_copy(stC[0:1, 0:CHUNK], psC_n[:])
            nc.scalar.copy(stC[0:1, CR:CR + CHUNK], psC_a[:])
            nc.vector.tensor_copy(stC[0:1, CHUNK:CR], psD_n[:])
            nc.scalar.copy(stC[0:1, CR + CHUNK:2 * CR], psD_a[:])
            nc.sync.dma_start(outC[0:1, 0:2 * CR], stC[:])

    nc.compile()
    return nc


def _get_compiled():
    global _COMPILED
    if _COMPILED is None:
        _COMPILED = _build()
    return _COMPILED


def kernel(encoded, normal_dist, anomaly_dist):
    global LAST_RESULTS
    from concourse.bass_utils import run_bass_kernel_spmd

    x = np.asarray(encoded, dtype=np.float32)
    nd = np.asarray(normal_dist, dtype=np.float64)
    ad = np.asarray(anomaly_dist, dtype=np.float64)

    # per-dim stats (torch defaults: unbiased std)
    mu_n = nd.mean(axis=1)
    sd_n = nd.std(axis=1, ddof=1)
    mu_a = ad.mean(axis=1)
    sd_a = ad.std(axis=1, ddof=1)
    isd_n, isd_a = 1.0 / sd_n, 1.0 / sd_a

    inv_sqrt2 = 1.0 / np.sqrt(2.0)
    base = np.stack([
        isd_n * inv_sqrt2,            # scale_n
        -mu_n * isd_n * inv_sqrt2,    # bias_n
        isd_a * inv_sqrt2,            # scale_a
        -mu_a * isd_a * inv_sqrt2,    # bias_a
    ], axis=1)
    # cols 4-7: same affine scaled by 1/4 for the DVE chain (z' = z/sqrt(32))
    consts = np.concatenate([base, base * 0.25], axis=1).astype(np.float32)

    half_sqrt_pi = 0.5 * np.sqrt(np.pi)
    c_n = (INV_SQRT_2PI * isd_n * half_sqrt_pi).astype(np.float16)
    c_a = (INV_SQRT_2PI * isd_a * half_sqrt_pi).astype(np.float16)
    wmat = np.zeros((DIM, 512), dtype=np.float16)
    wmat[:, 128] = c_n
    wmat[:, 384] = c_a

    in_maps = []
    for i in range(NCORES):
        lo = i * R
        shard_T = np.ascontiguousarray(x[lo:lo + R].T.astype(np.float16))
        in_maps.append({"xT": shard_T, "consts": consts, "wmat": wmat})

    nc = _get_compiled()
    try:
        res = run_bass_kernel_spmd(nc, in_maps, core_ids=list(range(NCORES)))
    except Exception:
        # one retry: the NRT occasionally reports a transient
        # NRT_EXEC_UNIT_UNRECOVERABLE on an otherwise-healthy device
        res = run_bass_kernel_spmd(nc, in_maps, core_ids=list(range(NCORES)))
    LAST_RESULTS = res

    s_n = np.empty(N, dtype=np.float64)
    s_a = np.empty(N, dtype=np.float64)
    for i in range(NCORES):
        lo = i * R
        a64 = res.results[i]["outA"][:, 0:1024].astype(np.float64)
        b57 = res.results[i]["outB"][0:57, 0:1024].astype(np.float64)
        cr = CHUNK + RUNT
        c1 = res.results[i]["outC"][0, 0:2 * cr].astype(np.float64)
        sn = np.concatenate([a64[:, 0:512].reshape(-1),
                             b57[:, 0:512].reshape(-1),
                             c1[0:cr]])
        sa = np.concatenate([a64[:, 512:1024].reshape(-1),
                             b57[:, 512:1024].reshape(-1),
                             c1[cr:2 * cr]])
        s_n[lo:lo + R] = sn
        s_a[lo:lo + R] = sa

    # exact recurrence p_k = (p_{k-1} + s_k)/dim as truncated causal
    # convolution: p_k = sum_j (1/dim)^(j+1) s_{k-j}; (1/128)^14 ~ 3e-30.
    a = 1.0 / DIM
    pn = np.zeros(N, dtype=np.float64)
    pa = np.zeros(N, dtype=np.float64)
    wgt = a
    for j in range(14):
        if j == 0:
            pn += wgt * s_n
            pa += wgt * s_a
        else:
            pn[j:] += wgt * s_n[:-j]
            pa[j:] += wgt * s_a[:-j]
        wgt *= a
    total = pn + pa
    out = np.empty((N, 2), dtype=np.float32)
    out[:, 0] = (pn / total).astype(np.float32)
    out[:, 1] = (pa / total).astype(np.float32)
    return out


# revision 29
# speedup vs baseline: 1.1863x; 1.1863x over previous
"""Trainium2 Bass kernel for nn_ClassificationLayer (Gaussian pdf-sum classifier).

Math:
  mu/sd per dim from tiny [128,10] reference sets (host, exact).
  Per row i: s_n[i] = sum_d INV_SQRT_2PI/sd_d * exp(-0.5*((x[i,d]-mu_d)/sd_d)^2)
  (same for anomaly), then the batch recurrence p_k = (p_{k-1} + s_k)/128,
  output = [pn/(pn+pa), pa/(pn+pa)].

Device strategy (8 cores, data-parallel over N, 62500 rows/core exactly):
  - Host transposes each core's row-shard to fp16 [128 dims, R rows]; per-dim
    constants become per-partition scale/bias for the ScalarEngine. fp16 input
    halves HBM traffic so the DMA stream (45us) hides under the ScalarEngine
    floor (2 activation passes over all elements ~ 105us).
  - One ACTIVATE per distribution per tile: Derivative_Erf(scale*x + bias)
    = (2/sqrt(pi)) * exp(-((x-mu)/sd)^2/2) -- the whole Gaussian in one pass.
    The framework auto-inserts the activation-table load right after the
    preamble, off the critical path.
  - Reduction over dims (partitions) via TensorEngine matvec (fp16 x fp16).
    The stationary operand is a 64-wide shifted window over a zero-padded
    fp16 weight buffer so chunk g's sums land in PSUM partition g%64 of bank
    g//64. Chunk 121 and the 36-col runt use 1-wide stationaries into tiny
    2-partition PSUM banks so the final drain is one short copy + one DMA.
  - Output DRAM rows are padded to 128KB stride: the simulated SDMA assigns
    engine = (dram_offset / 128KB) % 16, so 4KB-strided rows would all land
    on 2 of the 16 engines and serialize the drain.
  - The scalar recurrence decays by 1/128 per step, so it is re-run exactly
    on the gathered per-row sums on host as a short causal convolution.
"""

import numpy as np

N, DIM, S = 500000, 128, 10
INV_SQRT_2PI = 0.3989422804014327
NCORES = 8
R = N // NCORES                  # 62500 rows per core, exact
CHUNK = 512                      # rows per matvec (PSUM bank free-dim)
FULL_CHUNKS = R // CHUNK         # 122
RUNT = R - FULL_CHUNKS * CHUNK   # 36
# tile widths in chunks: small head tiles so the ScalarEngine starts before
# the first big DMA lands, wide tiles to amortize per-instruction overhead,
# small tail so the final drain chain is short. The runt's 36 cols ride on
# the last tile (548-col activations, separate matmul).
TILE_CHUNKS = [1, 2, 4, 8, 16, 24, 24, 24, 16, 2, 1]
assert sum(TILE_CHUNKS) == FULL_CHUNKS
MAX_W = max(TILE_CHUNKS) * CHUNK
PADW = 32768                     # fp32 cols per padded DRAM out row (128KB)

_COMPILED = None
LAST_RESULTS = None  # BassKernelResults of the most recent device run


def _build():
    import concourse.tile as tile
    from concourse import bacc, mybir

    nc = bacc.Bacc("TRN2", target_bir_lowering=False, debug=False,
                   num_devices=NCORES)

    xT = nc.dram_tensor("xT", [DIM, R], mybir.dt.float16,
                        kind="ExternalInput").ap()
    # consts: col 0 scale_n, 1 bias_n, 2 scale_a, 3 bias_a,
    #         cols 4-7 = cols 0-3 / 4 (DVE chain: z' = z/sqrt(32))
    consts = nc.dram_tensor("consts", [DIM, 8], mybir.dt.float32,
                            kind="ExternalInput").ap()
    # wmat: col 128-r of window [128-r, 192-r) must be the weight vector:
    # c_n at col 128, c_a at col 384; everything else exactly 0.
    wmat = nc.dram_tensor("wmat", [DIM, 512], mybir.dt.float16,
                          kind="ExternalInput").ap()
    # outA: bank A (chunks 0-63):   rows 0-63, [sn 512 | sa 512]
    # outB: bank B (chunks 64-120): rows 0-56 (57-63 zero), [sn 512 | sa 512]
    #   (64 rows, not 57: the sim spreads a DMA over
    #    largest-divisor-of-partition-count <= 16 engines, so 57 rows would
    #    crawl on 3 engines while 64 rows get all 16)
    # outC: chunk 121 + runt:       one row [sn 512 | sn 36 | sa 512 | sa 36]
    outA = nc.dram_tensor("outA", [64, PADW], mybir.dt.float32,
                          kind="ExternalOutput").ap()
    outB = nc.dram_tensor("outB", [64, PADW], mybir.dt.float32,
                          kind="ExternalOutput").ap()
    outC = nc.dram_tensor("outC", [1, PADW], mybir.dt.float32,
                          kind="ExternalOutput").ap()

    DErf = mybir.ActivationFunctionType.Derivative_Erf

    tile_first_chunk = []
    g = 0
    for tc_ in TILE_CHUNKS:
        tile_first_chunk.append(g)
        g += tc_
    FLUSH_A_TILE = next(i for i, f in enumerate(tile_first_chunk)
                        if f + TILE_CHUNKS[i] > 63)
    FLUSH_B_TILE = next(i for i, f in enumerate(tile_first_chunk)
                        if f + TILE_CHUNKS[i] > 120)
    LAST_TILE = len(TILE_CHUNKS) - 1

    with tile.TileContext(nc) as tc, \
         nc.allow_low_precision(reason="fp16 pdf terms; sums accumulate in fp32 PSUM"):
        with tc.tile_pool(name="cpool", bufs=1) as cpool, \
             tc.tile_pool(name="xpool", bufs=3) as xpool, \
             tc.tile_pool(name="epool", bufs=2) as epool, \
             tc.tile_pool(name="dpool", bufs=1) as dpool, \
             tc.tile_pool(name="pspool", bufs=1, space="PSUM") as pspool:

            # first x tile ahead of the (tiny) consts DMA: x0's transfer
            # dominates the first ACTIVATE's critical path
            consts_t = cpool.tile([DIM, 8], mybir.dt.float32)
            x_pre = {}
            for ti in (0, 1):
                w = TILE_CHUNKS[ti] * CHUNK
                off = tile_first_chunk[ti] * CHUNK
                x_t = xpool.tile([DIM, w], mybir.dt.float16, tag="x",
                                 padded_shape=[DIM, MAX_W],
                                 name=f"x_pre{ti}")
                nc.sync.dma_start(x_t[:], xT[:, off:off + w])
                x_pre[ti] = x_t
            nc.sync.dma_start(consts_t[:], consts[:, :])
            # Dummy activation gated only on the tiny consts DMA: without it
            # the auto-inserted table load attaches to the first real ACTIVATE
            # and ends up waiting on the x-tile DMA, delaying the stream.
            warm_t = cpool.tile([DIM, 1], mybir.dt.float32)
            nc.scalar.activation(warm_t[:], consts_t[:, 0:1], DErf,
                                 bias=0.0, scale=1.0)
            # weights via SWDGE so the Sync HWDGE queue is x-tiles only
            w_t = cpool.tile([DIM, 512], mybir.dt.float16)
            nc.gpsimd.dma_start(w_t[:], wmat[:, :])

            # per dist: bank A = chunks 0-63, bank B = chunks 64-120,
            # bank C = chunk 121 (row 0 sn, row 1 sa), bank D = runt
            sn_psA = pspool.tile([64, CHUNK], mybir.dt.float32)
            sn_psB = pspool.tile([64, CHUNK], mybir.dt.float32)
            sa_psA = pspool.tile([64, CHUNK], mybir.dt.float32)
            sa_psB = pspool.tile([64, CHUNK], mybir.dt.float32)
            # chunk 121 and runt: one single-row bank per dist (PE output
            # base partition must be 0/32/64, so they can't share a bank)
            psC_n = pspool.tile([1, CHUNK], mybir.dt.float32)
            psC_a = pspool.tile([1, CHUNK], mybir.dt.float32)
            psD_n = pspool.tile([1, RUNT], mybir.dt.float32)
            psD_a = pspool.tile([1, RUNT], mybir.dt.float32)

            stA = dpool.tile([64, 1024], mybir.dt.float32)
            stB = dpool.tile([64, 1024], mybir.dt.float32)
            # one partition row: [sn 512 | sn 36 | sa 512 | sa 36]
            stC = dpool.tile([1, 2 * (CHUNK + RUNT)], mybir.dt.float32)

            # DVE offload: for the big tiles the trailing DVE_W columns of
            # both distributions are computed on the otherwise-idle Vector
            # engine via exp(-u) ~ (deg-4 Taylor of exp(-u/16))^16, u clamped
            # at 12 (floor exp(-12), the tail contributes nothing anyway).
            # The two distributions' chains are interleaved step by step so
            # the ~400ns cross-instruction semaphore latency of one chain is
            # hidden under the other chain's execution.
            # offload sized so total engine activity stays under the sim's
            # clock-throttle threshold (more DVE work trips a global 1.2x
            # slowdown) and the Vector engine re-converges with the
            # ScalarEngine before the drain tiles
            DVE_W = {4: 512, 5: 1024, 6: 1024, 7: 1024, 8: 512}
            DVMAX = max(DVE_W.values())
            dvpool_tiles = [[dpool.tile([DIM, DVMAX], mybir.dt.float16,
                                        name=f"dv{j}_{k}") for j in range(3)]
                            for k in range(2)]
            MUL = mybir.AluOpType.mult
            ADD = mybir.AluOpType.add
            MIN = mybir.AluOpType.min
            C16 = float((2.0 / np.sqrt(np.pi)) ** (1.0 / 16.0))

            def dve_chains(x_t, aw, w, en_t, ea_t):
                dvw = w - aw
                xs = x_t[:, aw:w]
                ab = []
                for k, e_t in ((0, en_t), (1, ea_t)):
                    z_t, v_t, h_t = dvpool_tiles[k]
                    zs = z_t[:, 0:dvw]
                    vs = v_t[:, 0:dvw]
                    hs = h_t[:, 0:dvw]
                    sc = consts_t[:, 4 + 2 * k:5 + 2 * k]
                    bi = consts_t[:, 5 + 2 * k:6 + 2 * k]
                    ab.append((zs, vs, hs, sc, bi, e_t))
                steps = [
                    lambda zs, vs, hs, sc, bi, e:
                        nc.vector.tensor_scalar(zs, xs, sc, bi, MUL, ADD),
                    lambda zs, vs, hs, sc, bi, e:
                        nc.vector.tensor_tensor(vs, zs, zs, MUL),
                    lambda zs, vs, hs, sc, bi, e:
                        nc.vector.tensor_scalar(zs, vs, 0.75, None, MIN),
                    lambda zs, vs, hs, sc, bi, e:
                        nc.vector.tensor_scalar(hs, zs, 1.0 / 24, -1.0 / 6,
                                                MUL, ADD),
                    lambda zs, vs, hs, sc, bi, e:
                        nc.vector.tensor_tensor(vs, hs, zs, MUL),
                    lambda zs, vs, hs, sc, bi, e:
                        nc.vector.tensor_scalar(hs, vs, 0.5, None, ADD),
                    lambda zs, vs, hs, sc, bi, e:
                        nc.vector.tensor_tensor(vs, hs, zs, MUL),
                    lambda zs, vs, hs, sc, bi, e:
                        nc.vector.tensor_scalar(hs, vs, -1.0, None, ADD),
                    lambda zs, vs, hs, sc, bi, e:
                        nc.vector.tensor_tensor(vs, hs, zs, MUL),
                    # p0 scaled by (2/sqrt(pi))^(1/16) so p0^16 picks up the
                    # Derivative_Erf prefactor the weights divide out
                    lambda zs, vs, hs, sc, bi, e:
                        nc.vector.tensor_scalar(hs, vs, 1.0, C16, ADD, MUL),
                    lambda zs, vs, hs, sc, bi, e:
                        nc.vector.tensor_tensor(vs, hs, hs, MUL),
                    lambda zs, vs, hs, sc, bi, e:
                        nc.vector.tensor_tensor(hs, vs, vs, MUL),
                    lambda zs, vs, hs, sc, bi, e:
                        nc.vector.tensor_tensor(vs, hs, hs, MUL),
                    lambda zs, vs, hs, sc, bi, e:
                        nc.vector.tensor_tensor(e[:, aw:w], vs, vs, MUL),
                ]
                for step in steps:
                    for args in ab:
                        step(*args)

            g = 0
            for ti, tcn in enumerate(TILE_CHUNKS):
                w = tcn * CHUNK
                off = tile_first_chunk[ti] * CHUNK
                if ti == LAST_TILE:
                    w += RUNT     # runt columns ride on the last tile's acts
                if ti in x_pre:
                    x_t = x_pre[ti]
                else:
                    x_t = xpool.tile([DIM, w], mybir.dt.float16, tag="x",
                                     padded_shape=[DIM, MAX_W])
                    nc.sync.dma_start(x_t[:], xT[:, off:off + w])
                dvw = DVE_W.get(ti, 0)
                aw = w - dvw
                en_t = epool.tile([DIM, w], mybir.dt.float16, tag="en",
                                  padded_shape=[DIM, MAX_W])
                nc.scalar.activation(en_t[:, 0:aw], x_t[:, 0:aw], DErf,
                                     bias=consts_t[:, 1:2],
                                     scale=consts_t[:, 0:1])
                ea_t = epool.tile([DIM, w], mybir.dt.float16, tag="ea",
                                  padded_shape=[DIM, MAX_W])
                nc.scalar.activation(ea_t[:, 0:aw], x_t[:, 0:aw], DErf,
                                     bias=consts_t[:, 3:4],
                                     scale=consts_t[:, 2:3])
                if dvw:
                    dve_chains(x_t, aw, w, en_t, ea_t)
                for c in range(tcn):
                    sl = slice(c * CHUNK, (c + 1) * CHUNK)
                    r = g % 64
                    if g == 121:
                        # chunk 121: 1-wide stationaries into banks C
                        nc.tensor.matmul(psC_n[:], w_t[:, 128:129],
                                         en_t[:, sl], start=True, stop=True,
                                         skip_group_check=True)
                        nc.tensor.matmul(psC_a[:], w_t[:, 384:385],
                                         ea_t[:, sl], start=True, stop=True,
                                         skip_group_check=True)
                    else:
                        sn_ps = sn_psA if g < 64 else sn_psB
                        sa_ps = sa_psA if g < 64 else sa_psB
                        first = r == 0
                        last = g == 63 or g == 120
                        nc.tensor.matmul(sn_ps[:], w_t[:, 128 - r:192 - r],
                                         en_t[:, sl], start=first, stop=last,
                                         skip_group_check=True)
                        nc.tensor.matmul(sa_ps[:], w_t[:, 384 - r:448 - r],
                                         ea_t[:, sl], start=first, stop=last,
                                         skip_group_check=True)
                    g += 1
                if ti == LAST_TILE:
                    # runt: 36 cols through 1-wide stationaries into banks D
                    rsl = slice(tcn * CHUNK, tcn * CHUNK + RUNT)
                    nc.tensor.matmul(psD_n[:], w_t[:, 128:129],
                                     en_t[:, rsl], start=True, stop=True,
                                     skip_group_check=True)
                    nc.tensor.matmul(psD_a[:], w_t[:, 384:385],
                                     ea_t[:, rsl], start=True, stop=True,
                                     skip_group_check=True)
                if ti == FLUSH_A_TILE:
                    # bank A complete: drain it under the remaining compute
                    nc.vector.tensor_copy(stA[:, 0:512], sn_psA[:])
                    nc.vector.tensor_copy(stA[:, 512:1024], sa_psA[:])
                    nc.sync.dma_start(outA[:, 0:1024], stA[:])
                if ti == FLUSH_B_TILE:
                    # bank B rows 0-56 complete (57-63 zeroed by the start
                    # matmul): drain all 64 rows under the last tiles
                    nc.vector.tensor_copy(stB[:, 0:512], sn_psB[:])
                    nc.vector.tensor_copy(stB[:, 512:1024], sa_psB[:])
                    nc.sync.dma_start(outB[:, 0:1024], stB[:])

            # tail: short copies (sn on DVE, sa on the now-idle ScalarE in
            # parallel; GpSimd cannot read PSUM; engine writes must start at
            # partition 0/32/64/96 so everything lands in one partition row)
            CR = CHUNK + RUNT
            nc.vector.tensor_copy(stC[0:1, 0:CHUNK], psC_n[:])
            nc.scalar.copy(stC[0:1, CR:CR + CHUNK], psC_a[:])
            nc.vector.tensor_copy(stC[0:1, CHUNK:CR], psD_n[:])
            nc.scalar.copy(stC[0:1, CR + CHUNK:2 * CR], psD_a[:])
            nc.sync.dma_start(outC[0:1, 0:2 * CR], stC[:])

    nc.compile()
    return nc


def _get_compiled():
    global _COMPILED
    if _COMPILED is None:
        _COMPILED = _build()
    return _COMPILED


def kernel(encoded, normal_dist, anomaly_dist):
    global LAST_RESULTS
    from concourse.bass_utils import run_bass_kernel_spmd

    x = np.asarray(encoded, dtype=np.float32)
    nd = np.asarray(normal_dist, dtype=np.float64)
    ad = np.asarray(anomaly_dist, dtype=np.float64)

    # per-dim stats (torch defaults: unbiased std)
    mu_n = nd.mean(axis=1)
    sd_n = nd.std(axis=1, ddof=1)
    mu_a = ad.mean(axis=1)
    sd_a = ad.std(axis=1, ddof=1)
    isd_n, isd_a = 1.0 / sd_n, 1.0 / sd_a

    inv_sqrt2 = 1.0 / np.sqrt(2.0)
    base = np.stack([
        isd_n * inv_sqrt2,            # scale_n
        -mu_n * isd_n * inv_sqrt2,    # bias_n
        isd_a * inv_sqrt2,            # scale_a
        -mu_a * isd_a * inv_sqrt2,    # bias_a
    ], axis=1)
    # cols 4-7: same affine scaled by 1/4 for the DVE chain (z' = z/sqrt(32))
    consts = np.concatenate([base, base * 0.25], axis=1).astype(np.float32)

    half_sqrt_pi = 0.5 * np.sqrt(np.pi)
    c_n = (INV_SQRT_2PI * isd_n * half_sqrt_pi).astype(np.float16)
    c_a = (INV_SQRT_2PI * isd_a * half_sqrt_pi).astype(np.float16)
    wmat = np.zeros((DIM, 512), dtype=np.float16)
    wmat[:, 128] = c_n
    wmat[:, 384] = c_a

    in_maps = []
    for i in range(NCORES):
        lo = i * R
        shard_T = np.ascontiguousarray(x[lo:lo + R].T.astype(np.float16))
        in_maps.append({"xT": shard_T, "consts": consts, "wmat": wmat})

    nc = _get_compiled()
    try:
        res = run_bass_kernel_spmd(nc, in_maps, core_ids=list(range(NCORES)))
    except Exception:
        # one retry: the NRT occasionally reports a transient
        # NRT_EXEC_UNIT_UNRECOVERABLE on an otherwise-healthy device
        res = run_bass_kernel_spmd(nc, in_maps, core_ids=list(range(NCORES)))
    LAST_RESULTS = res

    s_n = np.empty(N, dtype=np.float64)
    s_a = np.empty(N, dtype=np.float64)
    for i in range(NCORES):
        lo = i * R
        a64 = res.results[i]["outA"][:, 0:1024].astype(np.float64)
        b57 = res.results[i]["outB"][0:57, 0:1024].astype(np.float64)
        cr = CHUNK + RUNT
        c1 = res.results[i]["outC"][0, 0:2 * cr].astype(np.float64)
        sn = np.concatenate([a64[:, 0:512].reshape(-1),
                             b57[:, 0:512].reshape(-1),
                             c1[0:cr]])
        sa = np.concatenate([a64[:, 512:1024].reshape(-1),
                             b57[:, 512:1024].reshape(-1),
                             c1[cr:2 * cr]])
        s_n[lo:lo + R] = sn
        s_a[lo:lo + R] = sa

    # exact recurrence p_k = (p_{k-1} + s_k)/dim as truncated causal
    # convolution: p_k = sum_j (1/dim)^(j+1) s_{k-j}; (1/128)^14 ~ 3e-30.
    a = 1.0 / DIM
    pn = np.zeros(N, dtype=np.float64)
    pa = np.zeros(N, dtype=np.float64)
    wgt = a
    for j in range(14):
        if j == 0:
            pn += wgt * s_n
            pa += wgt * s_a
        else:
            pn[j:] += wgt * s_n[:-j]
            pa[j:] += wgt * s_a[:-j]
        wgt *= a
    total = pn + pa
    out = np.empty((N, 2), dtype=np.float32)
    out[:, 0] = (pn / total).astype(np.float32)
    out[:, 1] = (pa / total).astype(np.float32)
    return out


# revision 30
# speedup vs baseline: 1.2013x; 1.0127x over previous
"""Trainium2 Bass kernel for nn_ClassificationLayer (Gaussian pdf-sum classifier).

Math:
  mu/sd per dim from tiny [128,10] reference sets (host, exact).
  Per row i: s_n[i] = sum_d INV_SQRT_2PI/sd_d * exp(-0.5*((x[i,d]-mu_d)/sd_d)^2)
  (same for anomaly), then the batch recurrence p_k = (p_{k-1} + s_k)/128,
  output = [pn/(pn+pa), pa/(pn+pa)].

Device strategy (8 cores, data-parallel over N, 62500 rows/core exactly):
  - Host transposes each core's row-shard to fp16 [128 dims, R rows]; per-dim
    constants become per-partition scale/bias for the ScalarEngine. fp16 input
    halves HBM traffic so the DMA stream (45us) hides under the ScalarEngine
    floor (2 activation passes over all elements ~ 105us).
  - One ACTIVATE per distribution per tile: Derivative_Erf(scale*x + bias)
    = (2/sqrt(pi)) * exp(-((x-mu)/sd)^2/2) -- the whole Gaussian in one pass.
    The framework auto-inserts the activation-table load right after the
    preamble, off the critical path.
  - Reduction over dims (partitions) via TensorEngine matvec (fp16 x fp16).
    The stationary operand is a 64-wide shifted window over a zero-padded
    fp16 weight buffer so chunk g's sums land in PSUM partition g%64 of bank
    g//64. Chunk 121 and the 36-col runt use 1-wide stationaries into tiny
    2-partition PSUM banks so the final drain is one short copy + one DMA.
  - Output DRAM rows are padded to 128KB stride: the simulated SDMA assigns
    engine = (dram_offset / 128KB) % 16, so 4KB-strided rows would all land
    on 2 of the 16 engines and serialize the drain.
  - The scalar recurrence decays by 1/128 per step, so it is re-run exactly
    on the gathered per-row sums on host as a short causal convolution.
"""

import numpy as np

N, DIM, S = 500000, 128, 10
INV_SQRT_2PI = 0.3989422804014327
NCORES = 8
R = N // NCORES                  # 62500 rows per core, exact
CHUNK = 512                      # rows per matvec (PSUM bank free-dim)
FULL_CHUNKS = R // CHUNK         # 122
RUNT = R - FULL_CHUNKS * CHUNK   # 36
# tile widths in chunks: small head tiles so the ScalarEngine starts before
# the first big DMA lands, wide tiles to amortize per-instruction overhead,
# small tail so the final drain chain is short. The runt's 36 cols ride on
# the last tile (548-col activations, separate matmul).
TILE_CHUNKS = [1, 2, 4, 8, 16, 24, 24, 24, 16, 2, 1]
assert sum(TILE_CHUNKS) == FULL_CHUNKS
MAX_W = max(TILE_CHUNKS) * CHUNK
PADW = 32768                     # fp32 cols per padded DRAM out row (128KB)

_COMPILED = None
LAST_RESULTS = None  # BassKernelResults of the most recent device run


def _build():
    import concourse.tile as tile
    from concourse import bacc, mybir

    nc = bacc.Bacc("TRN2", target_bir_lowering=False, debug=False,
                   num_devices=NCORES)

    xT = nc.dram_tensor("xT", [DIM, R], mybir.dt.float16,
                        kind="ExternalInput").ap()
    # consts: col 0 scale_n, 1 bias_n, 2 scale_a, 3 bias_a,
    #         cols 4-7 = cols 0-3 / 4 (DVE chain: z' = z/sqrt(32))
    consts = nc.dram_tensor("consts", [DIM, 8], mybir.dt.float32,
                            kind="ExternalInput").ap()
    # wmat: col 128-r of window [128-r, 192-r) must be the weight vector:
    # c_n at col 128, c_a at col 384; everything else exactly 0.
    wmat = nc.dram_tensor("wmat", [DIM, 512], mybir.dt.float16,
                          kind="ExternalInput").ap()
    # outA: bank A (chunks 0-63):   rows 0-63, [sn 512 | sa 512]
    # outB: bank B (chunks 64-120): rows 0-56 (57-63 zero), [sn 512 | sa 512]
    #   (64 rows, not 57: the sim spreads a DMA over
    #    largest-divisor-of-partition-count <= 16 engines, so 57 rows would
    #    crawl on 3 engines while 64 rows get all 16)
    # outC: chunk 121 + runt:       one row [sn 512 | sn 36 | sa 512 | sa 36]
    outA = nc.dram_tensor("outA", [64, PADW], mybir.dt.float32,
                          kind="ExternalOutput").ap()
    outB = nc.dram_tensor("outB", [64, PADW], mybir.dt.float32,
                          kind="ExternalOutput").ap()
    outC = nc.dram_tensor("outC", [1, PADW], mybir.dt.float32,
                          kind="ExternalOutput").ap()

    DErf = mybir.ActivationFunctionType.Derivative_Erf

    tile_first_chunk = []
    g = 0
    for tc_ in TILE_CHUNKS:
        tile_first_chunk.append(g)
        g += tc_
    FLUSH_A_TILE = next(i for i, f in enumerate(tile_first_chunk)
                        if f + TILE_CHUNKS[i] > 63)
    FLUSH_B_TILE = next(i for i, f in enumerate(tile_first_chunk)
                        if f + TILE_CHUNKS[i] > 120)
    LAST_TILE = len(TILE_CHUNKS) - 1

    with tile.TileContext(nc) as tc, \
         nc.allow_low_precision(reason="fp16 pdf terms; sums accumulate in fp32 PSUM"):
        with tc.tile_pool(name="cpool", bufs=1) as cpool, \
             tc.tile_pool(name="xpool", bufs=3) as xpool, \
             tc.tile_pool(name="epool", bufs=2) as epool, \
             tc.tile_pool(name="dpool", bufs=1) as dpool, \
             tc.tile_pool(name="pspool", bufs=1, space="PSUM") as pspool:

            # consts first (tiny) so the table-load dummy fires early, then
            # the first x tiles so the first ACTIVATE's data is in flight
            # during the preamble/table load
            consts_t = cpool.tile([DIM, 8], mybir.dt.float32)
            nc.sync.dma_start(consts_t[:], consts[:, :])
            x_pre = {}
            for ti in (0, 1, 2):
                w = TILE_CHUNKS[ti] * CHUNK
                off = tile_first_chunk[ti] * CHUNK
                x_t = xpool.tile([DIM, w], mybir.dt.float16, tag="x",
                                 padded_shape=[DIM, MAX_W],
                                 name=f"x_pre{ti}")
                nc.sync.dma_start(x_t[:], xT[:, off:off + w])
                x_pre[ti] = x_t
            # Dummy activation gated only on the tiny consts DMA: without it
            # the auto-inserted table load attaches to the first real ACTIVATE
            # and ends up waiting on the x-tile DMA, delaying the stream.
            warm_t = cpool.tile([DIM, 1], mybir.dt.float32)
            nc.scalar.activation(warm_t[:], consts_t[:, 0:1], DErf,
                                 bias=0.0, scale=1.0)
            # weights via SWDGE so the Sync HWDGE queue is x-tiles only
            w_t = cpool.tile([DIM, 512], mybir.dt.float16)
            nc.gpsimd.dma_start(w_t[:], wmat[:, :])

            # per dist: bank A = chunks 0-63, bank B = chunks 64-120,
            # bank C = chunk 121 (row 0 sn, row 1 sa), bank D = runt
            sn_psA = pspool.tile([64, CHUNK], mybir.dt.float32)
            sn_psB = pspool.tile([64, CHUNK], mybir.dt.float32)
            sa_psA = pspool.tile([64, CHUNK], mybir.dt.float32)
            sa_psB = pspool.tile([64, CHUNK], mybir.dt.float32)
            # chunk 121 and runt: one single-row bank per dist (PE output
            # base partition must be 0/32/64, so they can't share a bank)
            psC_n = pspool.tile([1, CHUNK], mybir.dt.float32)
            psC_a = pspool.tile([1, CHUNK], mybir.dt.float32)
            psD_n = pspool.tile([1, RUNT], mybir.dt.float32)
            psD_a = pspool.tile([1, RUNT], mybir.dt.float32)

            stA = dpool.tile([64, 1024], mybir.dt.float32)
            stB = dpool.tile([64, 1024], mybir.dt.float32)
            # one partition row: [sn 512 | sn 36 | sa 512 | sa 36]
            stC = dpool.tile([1, 2 * (CHUNK + RUNT)], mybir.dt.float32)

            # DVE offload: for the big tiles the trailing DVE_W columns of
            # both distributions are computed on the otherwise-idle Vector
            # engine via exp(-u) ~ (deg-4 Taylor of exp(-u/16))^16, u clamped
            # at 12 (floor exp(-12), the tail contributes nothing anyway).
            # The two distributions' chains are interleaved step by step so
            # the ~400ns cross-instruction semaphore latency of one chain is
            # hidden under the other chain's execution.
            # offload sized so total engine activity stays under the sim's
            # clock-throttle threshold (more DVE work trips a global 1.2x
            # slowdown) and the Vector engine re-converges with the
            # ScalarEngine before the drain tiles
            DVE_W = {4: 512, 5: 1024, 6: 1024, 7: 1024, 8: 512}
            DVMAX = max(DVE_W.values())
            dvpool_tiles = [[dpool.tile([DIM, DVMAX], mybir.dt.float16,
                                        name=f"dv{j}_{k}") for j in range(3)]
                            for k in range(2)]
            MUL = mybir.AluOpType.mult
            ADD = mybir.AluOpType.add
            MIN = mybir.AluOpType.min
            C16 = float((2.0 / np.sqrt(np.pi)) ** (1.0 / 16.0))

            def dve_chains(x_t, aw, w, en_t, ea_t):
                dvw = w - aw
                xs = x_t[:, aw:w]
                ab = []
                for k, e_t in ((0, en_t), (1, ea_t)):
                    z_t, v_t, h_t = dvpool_tiles[k]
                    zs = z_t[:, 0:dvw]
                    vs = v_t[:, 0:dvw]
                    hs = h_t[:, 0:dvw]
                    sc = consts_t[:, 4 + 2 * k:5 + 2 * k]
                    bi = consts_t[:, 5 + 2 * k:6 + 2 * k]
                    ab.append((zs, vs, hs, sc, bi, e_t))
                steps = [
                    lambda zs, vs, hs, sc, bi, e:
                        nc.vector.tensor_scalar(zs, xs, sc, bi, MUL, ADD),
                    lambda zs, vs, hs, sc, bi, e:
                        nc.vector.tensor_tensor(vs, zs, zs, MUL),
                    lambda zs, vs, hs, sc, bi, e:
                        nc.vector.tensor_scalar(zs, vs, 0.75, None, MIN),
                    lambda zs, vs, hs, sc, bi, e:
                        nc.vector.tensor_scalar(hs, zs, 1.0 / 24, -1.0 / 6,
                                                MUL, ADD),
                    lambda zs, vs, hs, sc, bi, e:
                        nc.vector.tensor_tensor(vs, hs, zs, MUL),
                    lambda zs, vs, hs, sc, bi, e:
                        nc.vector.tensor_scalar(hs, vs, 0.5, None, ADD),
                    lambda zs, vs, hs, sc, bi, e:
                        nc.vector.tensor_tensor(vs, hs, zs, MUL),
                    lambda zs, vs, hs, sc, bi, e:
                        nc.vector.tensor_scalar(hs, vs, -1.0, None, ADD),
                    lambda zs, vs, hs, sc, bi, e:
                        nc.vector.tensor_tensor(vs, hs, zs, MUL),
                    # p0 scaled by (2/sqrt(pi))^(1/16) so p0^16 picks up the
                    # Derivative_Erf prefactor the weights divide out
                    lambda zs, vs, hs, sc, bi, e:
                        nc.vector.tensor_scalar(hs, vs, 1.0, C16, ADD, MUL),
                    lambda zs, vs, hs, sc, bi, e:
                        nc.vector.tensor_tensor(vs, hs, hs, MUL),
                    lambda zs, vs, hs, sc, bi, e:
                        nc.vector.tensor_tensor(hs, vs, vs, MUL),
                    lambda zs, vs, hs, sc, bi, e:
                        nc.vector.tensor_tensor(vs, hs, hs, MUL),
                    lambda zs, vs, hs, sc, bi, e:
                        nc.vector.tensor_tensor(e[:, aw:w], vs, vs, MUL),
                ]
                for step in steps:
                    for args in ab:
                        step(*args)

            g = 0
            for ti, tcn in enumerate(TILE_CHUNKS):
                w = tcn * CHUNK
                off = tile_first_chunk[ti] * CHUNK
                if ti == LAST_TILE:
                    w += RUNT     # runt columns ride on the last tile's acts
                if ti in x_pre:
                    x_t = x_pre[ti]
                else:
                    x_t = xpool.tile([DIM, w], mybir.dt.float16, tag="x",
                                     padded_shape=[DIM, MAX_W])
                    nc.sync.dma_start(x_t[:], xT[:, off:off + w])
                dvw = DVE_W.get(ti, 0)
                aw = w - dvw
                en_t = epool.tile([DIM, w], mybir.dt.float16, tag="en",
                                  padded_shape=[DIM, MAX_W])
                nc.scalar.activation(en_t[:, 0:aw], x_t[:, 0:aw], DErf,
                                     bias=consts_t[:, 1:2],
                                     scale=consts_t[:, 0:1])
                ea_t = epool.tile([DIM, w], mybir.dt.float16, tag="ea",
                                  padded_shape=[DIM, MAX_W])
                nc.scalar.activation(ea_t[:, 0:aw], x_t[:, 0:aw], DErf,
                                     bias=consts_t[:, 3:4],
                                     scale=consts_t[:, 2:3])
                if dvw:
                    dve_chains(x_t, aw, w, en_t, ea_t)
                for c in range(tcn):
                    sl = slice(c * CHUNK, (c + 1) * CHUNK)
                    r = g % 64
                    if g == 121:
                        # chunk 121: 1-wide stationaries into banks C
                        nc.tensor.matmul(psC_n[:], w_t[:, 128:129],
                                         en_t[:, sl], start=True, stop=True,
                                         skip_group_check=True)
                        nc.tensor.matmul(psC_a[:], w_t[:, 384:385],
                                         ea_t[:, sl], start=True, stop=True,
                                         skip_group_check=True)
                    else:
                        sn_ps = sn_psA if g < 64 else sn_psB
                        sa_ps = sa_psA if g < 64 else sa_psB
                        first = r == 0
                        last = g == 63 or g == 120
                        nc.tensor.matmul(sn_ps[:], w_t[:, 128 - r:192 - r],
                                         en_t[:, sl], start=first, stop=last,
                                         skip_group_check=True)
                        nc.tensor.matmul(sa_ps[:], w_t[:, 384 - r:448 - r],
                                         ea_t[:, sl], start=first, stop=last,
                                         skip_group_check=True)
                    g += 1
                if ti == LAST_TILE:
                    # runt: 36 cols through 1-wide stationaries into banks D
                    rsl = slice(tcn * CHUNK, tcn * CHUNK + RUNT)
                    nc.tensor.matmul(psD_n[:], w_t[:, 128:129],
                                     en_t[:, rsl], start=True, stop=True,
                                     skip_group_check=True)
                    nc.tensor.matmul(psD_a[:], w_t[:, 384:385],
                                     ea_t[:, rsl], start=True, stop=True,
                                     skip_group_check=True)
                if ti == FLUSH_A_TILE:
                    # bank A complete: drain it under the remaining compute
                    nc.vector.tensor_copy(stA[:, 0:512], sn_psA[:])
                    nc.vector.tensor_copy(stA[:, 512:1024], sa_psA[:])
                    nc.sync.dma_start(outA[:, 0:1024], stA[:])
                if ti == FLUSH_B_TILE:
                    # bank B rows 0-56 complete (57-63 zeroed by the start
                    # matmul): drain all 64 rows under the last tiles
                    nc.vector.tensor_copy(stB[:, 0:512], sn_psB[:])
                    nc.vector.tensor_copy(stB[:, 512:1024], sa_psB[:])
                    nc.sync.dma_start(outB[:, 0:1024], stB[:])

            # tail: short copies (sn on DVE, sa on the now-idle ScalarE in
            # parallel; GpSimd cannot read PSUM; engine writes must start at
            # partition 0/32/64/96 so everything lands in one partition row)
            CR = CHUNK + RUNT
            nc.vector.tensor_copy(stC[0:1, 0:CHUNK], psC_n[:])
            nc.scalar.copy(stC[0:1, CR:CR + CHUNK], psC_a[:])
            nc.vector.tensor_copy(stC[0:1, CHUNK:CR], psD_n[:])
            nc.scalar.copy(stC[0:1, CR + CHUNK:2 * CR], psD_a[:])
            nc.sync.dma_start(outC[0:1, 0:2 * CR], stC[:])

    nc.compile()
    return nc


def _get_compiled():
    global _COMPILED
    if _COMPILED is None:
        _COMPILED = _build()
    return _COMPILED


def kernel(encoded, normal_dist, anomaly_dist):
    global LAST_RESULTS
    from concourse.bass_utils import run_bass_kernel_spmd

    x = np.asarray(encoded, dtype=np.float32)
    nd = np.asarray(normal_dist, dtype=np.float64)
    ad = np.asarray(anomaly_dist, dtype=np.float64)

    # per-dim stats (torch defaults: unbiased std)
    mu_n = nd.mean(axis=1)
    sd_n = nd.std(axis=1, ddof=1)
    mu_a = ad.mean(axis=1)
    sd_a = ad.std(axis=1, ddof=1)
    isd_n, isd_a = 1.0 / sd_n, 1.0 / sd_a

    inv_sqrt2 = 1.0 / np.sqrt(2.0)
    base = np.stack([
        isd_n * inv_sqrt2,            # scale_n
        -mu_n * isd_n * inv_sqrt2,    # bias_n
        isd_a * inv_sqrt2,            # scale_a
        -mu_a * isd_a * inv_sqrt2,    # bias_a
    ], axis=1)
    # cols 4-7: same affine scaled by 1/4 for the DVE chain (z' = z/sqrt(32))
    consts = np.concatenate([base, base * 0.25], axis=1).astype(np.float32)

    half_sqrt_pi = 0.5 * np.sqrt(np.pi)
    c_n = (INV_SQRT_2PI * isd_n * half_sqrt_pi).astype(np.float16)
    c_a = (INV_SQRT_2PI * isd_a * half_sqrt_pi).astype(np.float16)
    wmat = np.zeros((DIM, 512), dtype=np.float16)
    wmat[:, 128] = c_n
    wmat[:, 384] = c_a

    in_maps = []
    for i in range(NCORES):
        lo = i * R
        shard_T = np.ascontiguousarray(x[lo:lo + R].T.astype(np.float16))
        in_maps.append({"xT": shard_T, "consts": consts, "wmat": wmat})

    nc = _get_compiled()
    try:
        res = run_bass_kernel_spmd(nc, in_maps, core_ids=list(range(NCORES)))
    except Exception:
        # one retry: the NRT occasionally reports a transient
        # NRT_EXEC_UNIT_UNRECOVERABLE on an otherwise-healthy device
        res = run_bass_kernel_spmd(nc, in_maps, core_ids=list(range(NCORES)))
    LAST_RESULTS = res

    s_n = np.empty(N, dtype=np.float64)
    s_a = np.empty(N, dtype=np.float64)
    for i in range(NCORES):
        lo = i * R
        a64 = res.results[i]["outA"][:, 0:1024].astype(np.float64)
        b57 = res.results[i]["outB"][0:57, 0:1024].astype(np.float64)
        cr = CHUNK + RUNT
        c1 = res.results[i]["outC"][0, 0:2 * cr].astype(np.float64)
        sn = np.concatenate([a64[:, 0:512].reshape(-1),
                             b57[:, 0:512].reshape(-1),
                             c1[0:cr]])
        sa = np.concatenate([a64[:, 512:1024].reshape(-1),
                             b57[:, 512:1024].reshape(-1),
                             c1[cr:2 * cr]])
        s_n[lo:lo + R] = sn
        s_a[lo:lo + R] = sa

    # exact recurrence p_k = (p_{k-1} + s_k)/dim as truncated causal
    # convolution: p_k = sum_j (1/dim)^(j+1) s_{k-j}; (1/128)^14 ~ 3e-30.
    a = 1.0 / DIM
    pn = np.zeros(N, dtype=np.float64)
    pa = np.zeros(N, dtype=np.float64)
    wgt = a
    for j in range(14):
        if j == 0:
            pn += wgt * s_n
            pa += wgt * s_a
        else:
            pn[j:] += wgt * s_n[:-j]
            pa[j:] += wgt * s_a[:-j]
        wgt *= a
    total = pn + pa
    out = np.empty((N, 2), dtype=np.float32)
    out[:, 0] = (pn / total).astype(np.float32)
    out[:, 1] = (pa / total).astype(np.float32)
    return out


# revision 33
# speedup vs baseline: 1.2187x; 1.0145x over previous
"""Trainium2 Bass kernel for nn_ClassificationLayer (Gaussian pdf-sum classifier).

Math:
  mu/sd per dim from tiny [128,10] reference sets (host, exact).
  Per row i: s_n[i] = sum_d INV_SQRT_2PI/sd_d * exp(-0.5*((x[i,d]-mu_d)/sd_d)^2)
  (same for anomaly), then the batch recurrence p_k = (p_{k-1} + s_k)/128,
  output = [pn/(pn+pa), pa/(pn+pa)].

Device strategy (8 cores, data-parallel over N, 62500 rows/core exactly):
  - Host transposes each core's row-shard to fp16 [128 dims, R rows]; per-dim
    constants become per-partition scale/bias for the ScalarEngine. fp16 input
    halves HBM traffic so the DMA stream (45us) hides under the ScalarEngine
    floor (2 activation passes over all elements ~ 105us).
  - One ACTIVATE per distribution per tile: Derivative_Erf(scale*x + bias)
    = (2/sqrt(pi)) * exp(-((x-mu)/sd)^2/2) -- the whole Gaussian in one pass.
    The framework auto-inserts the activation-table load right after the
    preamble, off the critical path.
  - Reduction over dims (partitions) via TensorEngine matvec (fp16 x fp16).
    The stationary operand is a 64-wide shifted window over a zero-padded
    fp16 weight buffer so chunk g's sums land in PSUM partition g%64 of bank
    g//64. Chunk 121 and the 36-col runt use 1-wide stationaries into tiny
    2-partition PSUM banks so the final drain is one short copy + one DMA.
  - Output DRAM rows are padded to 128KB stride: the simulated SDMA assigns
    engine = (dram_offset / 128KB) % 16, so 4KB-strided rows would all land
    on 2 of the 16 engines and serialize the drain.
  - The scalar recurrence decays by 1/128 per step, so it is re-run exactly
    on the gathered per-row sums on host as a short causal convolution.
"""

import numpy as np

N, DIM, S = 500000, 128, 10
INV_SQRT_2PI = 0.3989422804014327
NCORES = 8
R = N // NCORES                  # 62500 rows per core, exact
CHUNK = 512                      # rows per matvec (PSUM bank free-dim)
FULL_CHUNKS = R // CHUNK         # 122
RUNT = R - FULL_CHUNKS * CHUNK   # 36
# tile widths in chunks: small head tiles so the ScalarEngine starts before
# the first big DMA lands, wide tiles to amortize per-instruction overhead,
# small tail so the final drain chain is short. The runt's 36 cols ride on
# the last tile (548-col activations, separate matmul).
TILE_CHUNKS = [1, 2, 4, 8, 24, 24, 24, 24, 10, 1]
assert sum(TILE_CHUNKS) == FULL_CHUNKS
MAX_W = max(TILE_CHUNKS) * CHUNK
PADW = 32768                     # fp32 cols per padded DRAM out row (128KB)

_COMPILED = None
LAST_RESULTS = None  # BassKernelResults of the most recent device run


def _build():
    import concourse.tile as tile
    from concourse import bacc, mybir

    nc = bacc.Bacc("TRN2", target_bir_lowering=False, debug=False,
                   num_devices=NCORES)

    xT = nc.dram_tensor("xT", [DIM, R], mybir.dt.float16,
                        kind="ExternalInput").ap()
    # consts: col 0 scale_n, 1 bias_n, 2 scale_a, 3 bias_a,
    #         cols 4-7 = cols 0-3 / 4 (DVE chain: z' = z/sqrt(32))
    consts = nc.dram_tensor("consts", [DIM, 8], mybir.dt.float32,
                            kind="ExternalInput").ap()
    # wmat: col 128-r of window [128-r, 192-r) must be the weight vector:
    # c_n at col 128, c_a at col 384; everything else exactly 0.
    wmat = nc.dram_tensor("wmat", [DIM, 512], mybir.dt.float16,
                          kind="ExternalInput").ap()
    # outA: bank A (chunks 0-63):   rows 0-63, [sn 512 | sa 512]
    # outB: bank B (chunks 64-120): rows 0-56 (57-63 zero), [sn 512 | sa 512]
    #   (64 rows, not 57: the sim spreads a DMA over
    #    largest-divisor-of-partition-count <= 16 engines, so 57 rows would
    #    crawl on 3 engines while 64 rows get all 16)
    # outC: chunk 121 + runt:       one row [sn 512 | sn 36 | sa 512 | sa 36]
    outA = nc.dram_tensor("outA", [64, PADW], mybir.dt.float32,
                          kind="ExternalOutput").ap()
    outB = nc.dram_tensor("outB", [64, PADW], mybir.dt.float32,
                          kind="ExternalOutput").ap()
    outC = nc.dram_tensor("outC", [1, PADW], mybir.dt.float32,
                          kind="ExternalOutput").ap()

    DErf = mybir.ActivationFunctionType.Derivative_Erf

    tile_first_chunk = []
    g = 0
    for tc_ in TILE_CHUNKS:
        tile_first_chunk.append(g)
        g += tc_
    FLUSH_A_TILE = next(i for i, f in enumerate(tile_first_chunk)
                        if f + TILE_CHUNKS[i] > 63)
    FLUSH_B_TILE = next(i for i, f in enumerate(tile_first_chunk)
                        if f + TILE_CHUNKS[i] > 120)
    LAST_TILE = len(TILE_CHUNKS) - 1

    with tile.TileContext(nc) as tc, \
         nc.allow_low_precision(reason="fp16 pdf terms; sums accumulate in fp32 PSUM"):
        with tc.tile_pool(name="cpool", bufs=1) as cpool, \
             tc.tile_pool(name="xpool", bufs=3) as xpool, \
             tc.tile_pool(name="epool", bufs=2) as epool, \
             tc.tile_pool(name="dpool", bufs=1) as dpool, \
             tc.tile_pool(name="pspool", bufs=1, space="PSUM") as pspool:

            # consts first (tiny) so the table-load dummy fires early, then
            # the first x tiles so the first ACTIVATE's data is in flight
            # during the preamble/table load
            consts_t = cpool.tile([DIM, 8], mybir.dt.float32)
            nc.sync.dma_start(consts_t[:], consts[:, :])
            x_pre = {}
            for ti in (0, 1, 2):
                w = TILE_CHUNKS[ti] * CHUNK
                off = tile_first_chunk[ti] * CHUNK
                x_t = xpool.tile([DIM, w], mybir.dt.float16, tag="x",
                                 padded_shape=[DIM, MAX_W],
                                 name=f"x_pre{ti}")
                nc.sync.dma_start(x_t[:], xT[:, off:off + w])
                x_pre[ti] = x_t
            # Dummy activation reading a framework-written const AP (no DMA
            # dependency at all): it fires right after the preamble so the
            # auto-inserted activation-table load is done before the first
            # x tile even lands.
            warm_t = cpool.tile([DIM, 1], mybir.dt.float32)
            zin = nc.const_aps.scalar_like(0.0, consts_t[:, 0:1])
            nc.scalar.activation(warm_t[:], zin, DErf,
                                 bias=0.0, scale=1.0)
            # weights via SWDGE so the Sync HWDGE queue is x-tiles only
            w_t = cpool.tile([DIM, 512], mybir.dt.float16)
            nc.gpsimd.dma_start(w_t[:], wmat[:, :])

            # per dist: bank A = chunks 0-63, bank B = chunks 64-120,
            # bank C = chunk 121 (row 0 sn, row 1 sa), bank D = runt
            sn_psA = pspool.tile([64, CHUNK], mybir.dt.float32)
            sn_psB = pspool.tile([64, CHUNK], mybir.dt.float32)
            sa_psA = pspool.tile([64, CHUNK], mybir.dt.float32)
            sa_psB = pspool.tile([64, CHUNK], mybir.dt.float32)
            # chunk 121 and runt: one single-row bank per dist (PE output
            # base partition must be 0/32/64, so they can't share a bank)
            psC_n = pspool.tile([1, CHUNK], mybir.dt.float32)
            psC_a = pspool.tile([1, CHUNK], mybir.dt.float32)
            psD_n = pspool.tile([1, RUNT], mybir.dt.float32)
            psD_a = pspool.tile([1, RUNT], mybir.dt.float32)

            stA = dpool.tile([64, 1024], mybir.dt.float32)
            stB = dpool.tile([64, 1024], mybir.dt.float32)
            # one partition row: [sn 512 | sn 36 | sa 512 | sa 36]
            stC = dpool.tile([1, 2 * (CHUNK + RUNT)], mybir.dt.float32)

            # DVE offload: for the big tiles the trailing DVE_W columns of
            # both distributions are computed on the otherwise-idle Vector
            # engine via exp(-u) ~ (deg-4 Taylor of exp(-u/16))^16, u clamped
            # at 12 (floor exp(-12), the tail contributes nothing anyway).
            # The two distributions' chains are interleaved step by step so
            # the ~400ns cross-instruction semaphore latency of one chain is
            # hidden under the other chain's execution.
            # offload sized so total engine activity stays under the sim's
            # clock-throttle threshold (more DVE work trips a global 1.2x
            # slowdown); only the 24-chunk tiles carry DVE work so the last
            # three tiles drain with the Vector engine already idle
            DVE_W = {4: 1024, 5: 1024, 6: 1024, 7: 1024}
            DVMAX = max(DVE_W.values())
            dvpool_tiles = [[dpool.tile([DIM, DVMAX], mybir.dt.float16,
                                        name=f"dv{j}_{k}") for j in range(3)]
                            for k in range(2)]
            MUL = mybir.AluOpType.mult
            ADD = mybir.AluOpType.add
            MIN = mybir.AluOpType.min
            C16 = float((2.0 / np.sqrt(np.pi)) ** (1.0 / 16.0))

            def dve_chains(x_t, aw, w, en_t, ea_t):
                dvw = w - aw
                xs = x_t[:, aw:w]
                ab = []
                for k, e_t in ((0, en_t), (1, ea_t)):
                    z_t, v_t, h_t = dvpool_tiles[k]
                    zs = z_t[:, 0:dvw]
                    vs = v_t[:, 0:dvw]
                    hs = h_t[:, 0:dvw]
                    sc = consts_t[:, 4 + 2 * k:5 + 2 * k]
                    bi = consts_t[:, 5 + 2 * k:6 + 2 * k]
                    ab.append((zs, vs, hs, sc, bi, e_t))
                steps = [
                    lambda zs, vs, hs, sc, bi, e:
                        nc.vector.tensor_scalar(zs, xs, sc, bi, MUL, ADD),
                    lambda zs, vs, hs, sc, bi, e:
                        nc.vector.tensor_tensor(vs, zs, zs, MUL),
                    lambda zs, vs, hs, sc, bi, e:
                        nc.vector.tensor_scalar(zs, vs, 0.75, None, MIN),
                    lambda zs, vs, hs, sc, bi, e:
                        nc.vector.tensor_scalar(hs, zs, 1.0 / 24, -1.0 / 6,
                                                MUL, ADD),
                    lambda zs, vs, hs, sc, bi, e:
                        nc.vector.tensor_tensor(vs, hs, zs, MUL),
                    lambda zs, vs, hs, sc, bi, e:
                        nc.vector.tensor_scalar(hs, vs, 0.5, None, ADD),
                    lambda zs, vs, hs, sc, bi, e:
                        nc.vector.tensor_tensor(vs, hs, zs, MUL),
                    lambda zs, vs, hs, sc, bi, e:
                        nc.vector.tensor_scalar(hs, vs, -1.0, None, ADD),
                    lambda zs, vs, hs, sc, bi, e:
                        nc.vector.tensor_tensor(vs, hs, zs, MUL),
                    # p0 scaled by (2/sqrt(pi))^(1/16) so p0^16 picks up the
                    # Derivative_Erf prefactor the weights divide out
                    lambda zs, vs, hs, sc, bi, e:
                        nc.vector.tensor_scalar(hs, vs, 1.0, C16, ADD, MUL),
                    lambda zs, vs, hs, sc, bi, e:
                        nc.vector.tensor_tensor(vs, hs, hs, MUL),
                    lambda zs, vs, hs, sc, bi, e:
                        nc.vector.tensor_tensor(hs, vs, vs, MUL),
                    lambda zs, vs, hs, sc, bi, e:
                        nc.vector.tensor_tensor(vs, hs, hs, MUL),
                    lambda zs, vs, hs, sc, bi, e:
                        nc.vector.tensor_tensor(e[:, aw:w], vs, vs, MUL),
                ]
                for step in steps:
                    for args in ab:
                        step(*args)

            g = 0
            for ti, tcn in enumerate(TILE_CHUNKS):
                w = tcn * CHUNK
                off = tile_first_chunk[ti] * CHUNK
                if ti == LAST_TILE:
                    w += RUNT     # runt columns ride on the last tile's acts
                if ti in x_pre:
                    x_t = x_pre[ti]
                else:
                    x_t = xpool.tile([DIM, w], mybir.dt.float16, tag="x",
                                     padded_shape=[DIM, MAX_W])
                    nc.sync.dma_start(x_t[:], xT[:, off:off + w])
                dvw = DVE_W.get(ti, 0)
                aw = w - dvw
                en_t = epool.tile([DIM, w], mybir.dt.float16, tag="en",
                                  padded_shape=[DIM, MAX_W])
                nc.scalar.activation(en_t[:, 0:aw], x_t[:, 0:aw], DErf,
                                     bias=consts_t[:, 1:2],
                                     scale=consts_t[:, 0:1])
                ea_t = epool.tile([DIM, w], mybir.dt.float16, tag="ea",
                                  padded_shape=[DIM, MAX_W])
                nc.scalar.activation(ea_t[:, 0:aw], x_t[:, 0:aw], DErf,
                                     bias=consts_t[:, 3:4],
                                     scale=consts_t[:, 2:3])
                if dvw:
                    dve_chains(x_t, aw, w, en_t, ea_t)
                for c in range(tcn):
                    sl = slice(c * CHUNK, (c + 1) * CHUNK)
                    r = g % 64
                    if g == 121:
                        # chunk 121: 1-wide stationaries into banks C
                        nc.tensor.matmul(psC_n[:], w_t[:, 128:129],
                                         en_t[:, sl], start=True, stop=True,
                                         skip_group_check=True)
                        nc.tensor.matmul(psC_a[:], w_t[:, 384:385],
                                         ea_t[:, sl], start=True, stop=True,
                                         skip_group_check=True)
                    else:
                        sn_ps = sn_psA if g < 64 else sn_psB
                        sa_ps = sa_psA if g < 64 else sa_psB
                        first = r == 0
                        last = g == 63 or g == 120
                        nc.tensor.matmul(sn_ps[:], w_t[:, 128 - r:192 - r],
                                         en_t[:, sl], start=first, stop=last,
                                         skip_group_check=True)
                        nc.tensor.matmul(sa_ps[:], w_t[:, 384 - r:448 - r],
                                         ea_t[:, sl], start=first, stop=last,
                                         skip_group_check=True)
                    g += 1
                if ti == LAST_TILE:
                    # runt: 36 cols through 1-wide stationaries into banks D
                    rsl = slice(tcn * CHUNK, tcn * CHUNK + RUNT)
                    nc.tensor.matmul(psD_n[:], w_t[:, 128:129],
                                     en_t[:, rsl], start=True, stop=True,
                                     skip_group_check=True)
                    nc.tensor.matmul(psD_a[:], w_t[:, 384:385],
                                     ea_t[:, rsl], start=True, stop=True,
                                     skip_group_check=True)
                if ti == FLUSH_A_TILE:
                    # bank A complete: drain it under the remaining compute
                    nc.vector.tensor_copy(stA[:, 0:512], sn_psA[:])
                    nc.vector.tensor_copy(stA[:, 512:1024], sa_psA[:])
                    nc.sync.dma_start(outA[:, 0:1024], stA[:])
                if ti == FLUSH_B_TILE:
                    # bank B rows 0-56 complete (57-63 zeroed by the start
                    # matmul): drain all 64 rows under the last tiles
                    nc.vector.tensor_copy(stB[:, 0:512], sn_psB[:])
                    nc.vector.tensor_copy(stB[:, 512:1024], sa_psB[:])
                    nc.sync.dma_start(outB[:, 0:1024], stB[:])

            # tail: short copies (sn on DVE, sa on the now-idle ScalarE in
            # parallel; GpSimd cannot read PSUM; engine writes must start at
            # partition 0/32/64/96 so everything lands in one partition row)
            CR = CHUNK + RUNT
            nc.vector.tensor_copy(stC[0:1, 0:CHUNK], psC_n[:])
            nc.scalar.copy(stC[0:1, CR:CR + CHUNK], psC_a[:])
            nc.vector.tensor_copy(stC[0:1, CHUNK:CR], psD_n[:])
            nc.scalar.copy(stC[0:1, CR + CHUNK:2 * CR], psD_a[:])
            nc.sync.dma_start(outC[0:1, 0:2 * CR], stC[:])

    nc.compile()
    return nc


def _get_compiled():
    global _COMPILED
    if _COMPILED is None:
        _COMPILED = _build()
    return _COMPILED


def kernel(encoded, normal_dist, anomaly_dist):
    global LAST_RESULTS
    from concourse.bass_utils import run_bass_kernel_spmd

    x = np.asarray(encoded, dtype=np.float32)
    nd = np.asarray(normal_dist, dtype=np.float64)
    ad = np.asarray(anomaly_dist, dtype=np.float64)

    # per-dim stats (torch defaults: unbiased std)
    mu_n = nd.mean(axis=1)
    sd_n = nd.std(axis=1, ddof=1)
    mu_a = ad.mean(axis=1)
    sd_a = ad.std(axis=1, ddof=1)
    isd_n, isd_a = 1.0 / sd_n, 1.0 / sd_a

    inv_sqrt2 = 1.0 / np.sqrt(2.0)
    base = np.stack([
        isd_n * inv_sqrt2,            # scale_n
        -mu_n * isd_n * inv_sqrt2,    # bias_n
        isd_a * inv_sqrt2,            # scale_a
        -mu_a * isd_a * inv_sqrt2,    # bias_a
    ], axis=1)
    # cols 4-7: same affine scaled by 1/4 for the DVE chain (z' = z/sqrt(32))
    consts = np.concatenate([base, base * 0.25], axis=1).astype(np.float32)

    half_sqrt_pi = 0.5 * np.sqrt(np.pi)
    c_n = (INV_SQRT_2PI * isd_n * half_sqrt_pi).astype(np.float16)
    c_a = (INV_SQRT_2PI * isd_a * half_sqrt_pi).astype(np.float16)
    wmat = np.zeros((DIM, 512), dtype=np.float16)
    wmat[:, 128] = c_n
    wmat[:, 384] = c_a

    in_maps = []
    for i in range(NCORES):
        lo = i * R
        shard_T = np.ascontiguousarray(x[lo:lo + R].T.astype(np.float16))
        in_maps.append({"xT": shard_T, "consts": consts, "wmat": wmat})

    nc = _get_compiled()
    try:
        res = run_bass_kernel_spmd(nc, in_maps, core_ids=list(range(NCORES)))
    except Exception:
        # one retry: the NRT occasionally reports a transient
        # NRT_EXEC_UNIT_UNRECOVERABLE on an otherwise-healthy device
        res = run_bass_kernel_spmd(nc, in_maps, core_ids=list(range(NCORES)))
    LAST_RESULTS = res

    s_n = np.empty(N, dtype=np.float64)
    s_a = np.empty(N, dtype=np.float64)
    for i in range(NCORES):
        lo = i * R
        a64 = res.results[i]["outA"][:, 0:1024].astype(np.float64)
        b57 = res.results[i]["outB"][0:57, 0:1024].astype(np.float64)
        cr = CHUNK + RUNT
        c1 = res.results[i]["outC"][0, 0:2 * cr].astype(np.float64)
        sn = np.concatenate([a64[:, 0:512].reshape(-1),
                             b57[:, 0:512].reshape(-1),
                             c1[0:cr]])
        sa = np.concatenate([a64[:, 512:1024].reshape(-1),
                             b57[:, 512:1024].reshape(-1),
                             c1[cr:2 * cr]])
        s_n[lo:lo + R] = sn
        s_a[lo:lo + R] = sa

    # exact recurrence p_k = (p_{k-1} + s_k)/dim as truncated causal
    # convolution: p_k = sum_j (1/dim)^(j+1) s_{k-j}; (1/128)^14 ~ 3e-30.
    a = 1.0 / DIM
    pn = np.zeros(N, dtype=np.float64)
    pa = np.zeros(N, dtype=np.float64)
    wgt = a
    for j in range(14):
        if j == 0:
            pn += wgt * s_n
            pa += wgt * s_a
        else:
            pn[j:] += wgt * s_n[:-j]
            pa[j:] += wgt * s_a[:-j]
        wgt *= a
    total = pn + pa
    out = np.empty((N, 2), dtype=np.float32)
    out[:, 0] = (pn / total).astype(np.float32)
    out[:, 1] = (pa / total).astype(np.float32)
    return out
